# revision 60
# baseline (speedup 1.0000x reference)
"""Multi-head attention (B=2, T=2048, D=1024, 16 heads) on 8 TRN2 NeuronCores.

Sharding: tensor-parallel over heads (2 heads/core). Each core computes
Q/K/V projections for its 2 heads (full sequence), causal attention in
the S^T = K @ Q^T form (so attn @ V needs no transposes), and a partial
output projection o_c = attn_out_c @ Wo[:, cols_c].T in bf16. The host
sums the 8 partial [4096, 1024] outputs (the tensor-parallel all-reduce
done on host) and reshapes to [2, 2048, 1024].

v2 changes vs baseline:
- softmax denominator: reciprocal via on-chip partition-broadcast
  (two K=1 matmuls into a PSUM bank) + reciprocal_approx_fast on all
  lanes + vector multiply; no DRAM bounce, no broadcast DMAs.
- causal-diagonal tiles narrow their score matmul / exp / AV-accum /
  mask-add windows to the unmasked column range (mask adds shrink from
  [128,2,512] to [128,2,128] for 3 of every 4 diagonal tiles).
- o partials written in bf16 (halves the 16MB output stream and the
  PSUM-evacuation copy).
- weights loaded as whole-tensor contiguous DMAs ordered so the first
  projection matmul can start ~3us in; V-ones columns via memset
  instead of a 2MB vinit DMA.
- engine rebalance: mask adds / qT,kT evacuations / normalize on
  Vector, exp + vtf evacuation on Scalar, V-transposes in f32r.
"""

import sys

sys.path.insert(0, "/opt/trn_rl_repo")

import numpy as np

B, T, D = 2, 2048, 1024
NCORES = 8
DV = 128  # head dims per core (2 heads x 64)
DH = 64
BT = B * T
CH = 512  # tq chunk width
NCH = BT // CH  # 8 global chunks
NCH_B = T // CH  # 4 chunks per batch
TK = 128  # tk tile
NTK = T // TK  # 16 tiles per batch
ND = D // 128  # 8 contraction tiles
DVA = DH + 1  # V columns incl ones
NEG = -1.0e30
MW = 256  # max mask block width

_cache = {}


def _build(cats_key, n_partial):
    """Build + compile the SPMD Bass kernel for a given mask block structure.

    cats_key: tuple over (jj, i) of 'f' (full), 's' (skip), or a tuple
    (midx, c0, wa, wb): partial with mask block midx, score/exp window
    starting at column c0, mask add window [wa, wb).
    """
    import concourse.bacc as bacc
    import concourse.mybir as mybir
    import concourse.tile as tile
    from concourse.masks import make_identity

    F32 = mybir.dt.float32
    F32R = mybir.dt.float32r
    BF16 = mybir.dt.bfloat16
    FP8 = mybir.dt.float8e5  # unused
    EXP = mybir.ActivationFunctionType.Exp
    COPYF = mybir.ActivationFunctionType.Copy
    MULT = mybir.AluOpType.mult
    ADD = mybir.AluOpType.add
    DR = mybir.MatmulPerfMode.DoubleRow
    # exp() is biased by -ln(16) so p fits fp8e4m3 (max 448) for scores up
    # to ~8.9; the scale cancels between the AV numerator and the ones-row
    # denominator.
    EXPB = -2.772588722239781

    cats = {}
    idx = 0
    for jj in range(NCH_B):
        for i in range(NTK):
            cats[(jj, i)] = cats_key[idx]
            idx += 1

    nc = bacc.Bacc("TRN2", target_bir_lowering=False, debug=False, num_devices=NCORES)

    xt_d = nc.dram_tensor("xt", [D, BT], F32R, kind="ExternalInput").ap()
    wq_d = nc.dram_tensor("wq", [128, D], F32R, kind="ExternalInput").ap()
    wk_d = nc.dram_tensor("wk", [128, D], F32R, kind="ExternalInput").ap()
    wv_d = nc.dram_tensor("wv", [128, D], F32R, kind="ExternalInput").ap()
    wo_d = nc.dram_tensor("wo", [128, D], F32R, kind="ExternalInput").ap()
    nmask = max(n_partial, 1)
    mask_d = nc.dram_tensor("mask", [nmask, 128, MW], BF16, kind="ExternalInput").ap()
    bc_d = nc.dram_tensor("bc", [1, 128], F32R, kind="ExternalInput").ap()
    o_d = nc.dram_tensor("o", [BT, D], BF16, kind="ExternalOutput").ap()

    with tile.TileContext(nc) as tc:
        with tc.tile_pool(name="consts", bufs=1) as consts, \
             tc.tile_pool(name="perm", bufs=1) as perm, \
             tc.tile_pool(name="xt_pool", bufs=6) as xtp, \
             tc.tile_pool(name="vtf_pool", bufs=3) as vtfp, \
             tc.tile_pool(name="p_pool", bufs=4) as ppool, \
             tc.tile_pool(name="outT_pool", bufs=3) as outTp, \
             tc.tile_pool(name="rec_pool", bufs=3) as recp, \
             tc.tile_pool(name="osb_pool", bufs=3) as obp, \
             tc.tile_pool(name="sps_ps", bufs=3, space="PSUM") as spsp, \
             tc.tile_pool(name="av_ps", bufs=1, space="PSUM") as avp, \
             tc.tile_pool(name="dram_pool", bufs=2, space="DRAM") as drp:
            wq_sb = consts.tile([128, D], F32R, name="wq_sb")
            wk_sb = consts.tile([128, D], F32R, name="wk_sb")
            wv_sb = consts.tile([128, D], F32R, name="wv_sb")
            wo_sb = consts.tile([128, D], F32R, name="wo_sb")
            ident = consts.tile([128, 128], F32, name="ident")
            ones1 = consts.tile([1, 128], F32R, name="ones1")
            ones64 = consts.tile([128, 64, 1], F32, name="ones64")
            expb = consts.tile([128, 1], F32, name="expb")
            mask_sb = consts.tile([128, nmask, MW], BF16, name="mask_sb")
            identB = consts.tile([128, 128], BF16, name="identB")
            make_identity(nc, ident[:])
            nc.scalar.activation(identB[:], ident[:], COPYF)
            nc.gpsimd.memset(ones64[:], 1.0)
            nc.gpsimd.memset(expb[:], EXPB)

            qT = perm.tile([128, BT], F32R, name="qT")
            kT = perm.tile([128, BT], F32R, name="kT")
            # V blocks in fp8, DoubleRow pair-interleaved: per (tile-pair pi,
            # head h, tile-in-pair j) a [128(tk), 65] block (64 dims + ones
            # col). DR lhsT slice is vsb8[:, pi, h, :, :] = [128, 2, 65].
            NPAIR = B * NTK // 2
            vsb8 = perm.tile([128, NPAIR, 2, 2, DVA], F32R, name="vsb8")
            nc.scalar.activation(
                vsb8[:].rearrange("p a h j d -> p (a h j) d")[:, :, DH:DVA],
                ones64[:], COPYF)

            # warm the PE while the first DMAs are in flight: a few cheap
            # fp32 matmuls on the identity ramp the p-state before xt0 lands
            warm = spsp.tile([128, 2, CH], F32, tag="sps", name="warm")
            for _ in range(5):
                nc.tensor.matmul(warm[:, 0, 0:128], ident[:], ident[:],
                                 start=True, stop=True)

            deferred = []

            def emit_oproj(b, jj, outT):
                for tt in range(4):
                    def step(tt=tt, b=b, jj=jj, outT=outT):
                        op = spsp.tile([128, 2, CH], F32, tag="sps",
                                       name=f"op{b}_{jj}_{tt}")
                        ts = slice(tt * 128, (tt + 1) * 128)
                        nc.tensor.matmul(op[:, 0, :], outT[:, ts],
                                         wo_sb[:, 0:CH], start=True, stop=True)
                        nc.tensor.matmul(op[:, 1, :], outT[:, ts],
                                         wo_sb[:, CH:D], start=True, stop=True)
                        osb = obp.tile([128, D], BF16, tag="osb",
                                       name=f"osb{b}_{jj}_{tt}")
                        nc.vector.tensor_copy(
                            osb[:].rearrange("p (a b) -> p a b", a=2), op[:])
                        r0 = b * T + jj * CH + tt * 128
                        nc.sync.dma_start(o_d[r0:r0 + 128, :], osb[:])
                    deferred.append(step)

            norm_tail = []
            projq = []

            def pump(n):
                for _ in range(n):
                    if projq:
                        projq.pop(0)()

            def attention_chunk(b, jj):
                kept = [i for i in range(NTK) if cats[(jj, i)] != 's']
                if not kept:
                    return
                av = avp.tile([128, 2, CH], F32, tag="av", name=f"av_{b}_{jj}")
                tq0 = (b * NCH_B + jj) * CH
                pend = None

                def c0_of(i):
                    c = cats[(jj, i)]
                    if c == 'f' or i == kept[0]:
                        return 0
                    return c[1]

                def emit_av(unit):
                    kind, tiles, p, c0 = unit
                    st = kept[0] in tiles
                    sp = kept[-1] in tiles
                    if kind == 'dr':
                        pi = (b * NTK + tiles[0]) // 2
                        for h in (0, 1):
                            nc.tensor.matmul(
                                av[0:DVA, h, :], vsb8[:, pi, h, :, 0:DVA],
                                p[:, h, :, :], start=st, stop=sp,
                                perf_mode=DR)
                    else:
                        u = b * NTK + tiles[0]
                        for h in (0, 1):
                            nc.tensor.matmul(
                                av[0:DVA, h, c0:CH],
                                vsb8[:, u // 2, h, u % 2, 0:DVA],
                                p[:, h, c0:CH], start=st, stop=sp)

                pairs = [kept[x:x + 2] for x in range(0, len(kept), 2)]
                for pidx, pair in enumerate(pairs):
                    dr = False and (len(pair) == 2 and pair[1] == pair[0] + 1
                          and pair[0] % 2 == 0
                          and all(cats[(jj, i)] == 'f' for i in pair))
                    group = []
                    for i in pair:
                        c0 = c0_of(i)
                        ks = slice((b * NTK + i) * TK, (b * NTK + i + 1) * TK)
                        tqs = slice(tq0 + c0, tq0 + CH)
                        sps = spsp.tile([128, 2, CH], F32, tag="sps",
                                        name=f"sps{b}_{jj}_{i}")
                        c = cats[(jj, i)]
                        masked = c != 'f' and c[0] >= 0
                        for h in (0, 1):
                            nc.tensor.matmul(sps[:, h, c0:CH],
                                             kT[h * 64:(h + 1) * 64, ks],
                                             qT[h * 64:(h + 1) * 64, tqs],
                                             start=True, stop=not masked)
                        if masked:
                            # additive causal mask as a PE matmul:
                            # sps[:, h, wa:wb] += I^T @ maskblock (bf16)
                            midx, _, wa, wb = c
                            for h in (0, 1):
                                nc.tensor.matmul(
                                    sps[:, h, wa:wb], identB[:],
                                    mask_sb[:, midx, 0:wb - wa],
                                    start=False, stop=True)
                        group.append((i, sps, c0))
                    units = []
                    if dr:
                        p8 = ppool.tile([128, 2, 2, CH], F32R, tag="p8",
                                        name=f"p8_{b}_{jj}_{pair[0]}")
                        for (i, sps, c0), j in zip(group, (0, 1)):
                            nc.scalar.activation(p8[:, :, j, :], sps[:],
                                                 EXP)
                        units.append(('dr', pair, p8, 0))
                    else:
                        for i, sps, c0 in group:
                            p = ppool.tile([128, 2, CH], F32R, tag="p1",
                                           name=f"p{b}_{jj}_{i}")
                            nc.scalar.activation(p[:, :, c0:CH],
                                                 sps[:, :, c0:CH],
                                                 EXP)
                            units.append(('sg', (i,), p, c0))
                    if norm_tail and pidx == 0:
                        norm_tail.pop(0)()
                    if deferred and pidx >= 1:
                        deferred.pop(0)()
                        if len(deferred) > 4:
                            deferred.pop(0)()
                    if pend is not None:
                        for unit in pend:
                            emit_av(unit)
                    pend = units
                    pump(2)
                for unit in pend:
                    emit_av(unit)

                def norm_step(b=b, jj=jj, av=av):
                    # normalization: den rows live at partition 64 of both
                    # av banks; broadcast across partitions with two K=1
                    # ones-matmuls, then approx-reciprocal on all lanes
                    su = recp.tile([1, 2, CH], F32R, tag="su",
                                   name=f"su_{b}_{jj}")
                    nc.vector.tensor_copy(su[:], av[DH:DVA, :, :])
                    denb = spsp.tile([128, 2, CH], F32, tag="sps",
                                     name=f"denb_{b}_{jj}")
                    nc.tensor.matmul(denb[:, 0, :], ones1[:], su[:, 0, :],
                                     start=True, stop=True)
                    nc.tensor.matmul(denb[:, 1, :], ones1[:], su[:, 1, :],
                                     start=True, stop=True)
                    rdc = recp.tile([128, 2, CH], F32, tag="rdc",
                                    name=f"rdc{b}_{jj}")
                    nc.vector.reciprocal_approx_fast(out=rdc[:],
                                                     in_=denb[:])
                    outT = outTp.tile([128, CH], F32R, tag="outT",
                                      name=f"outT{b}_{jj}")
                    for h in (0, 1):
                        nc.vector.tensor_tensor(
                            out=outT[h * 64:(h + 1) * 64, :],
                            in0=av[0:DH, h, :],
                            in1=rdc[h * 64:(h + 1) * 64, h, :], op=MULT)
                    emit_oproj(b, jj, outT)
                norm_tail.append(norm_step)

            # ------- projection for chunk-pair jp, emitted as steps that
            # interleave into the previous pair's attention ----------------
            def gen_proj(jp):
                j0 = 2 * jp
                acc = {}
                xts = {}
                steps = []

                def dma_xt(d, jp=jp, j0=j0):
                    xt = xtp.tile([128, 2 * CH], F32R, tag="xt",
                                  name=f"xt{jp}_{d}")
                    nc.sync.dma_start(
                        xt[:], xt_d[d * 128:(d + 1) * 128,
                                    j0 * CH:(j0 + 2) * CH])
                    xts[d] = xt

                def mm_step(d, jp=jp):
                    if d + 2 < ND:
                        dma_xt(d + 2)
                    if d == 0:
                        for nm in ("q", "k", "v"):
                            acc[nm] = spsp.tile([128, 2, CH], F32, tag="sps",
                                                name=f"{nm}ps{jp}")
                    st, sp = d == 0, d == ND - 1
                    ws = slice(d * 128, (d + 1) * 128)
                    for nm, w_sb in (("q", wq_sb), ("k", wk_sb),
                                     ("v", wv_sb)):
                        for half in (0, 1):
                            nc.tensor.matmul(
                                acc[nm][:, half, :], w_sb[:, ws],
                                xts[d][:, half * CH:(half + 1) * CH],
                                start=st, stop=sp)
                    xts.pop(d)

                def evac_qk(jp=jp, j0=j0):
                    cs = slice(j0 * CH, (j0 + 2) * CH)
                    nc.vector.tensor_copy(
                        qT[:, cs].rearrange("p (a b) -> p a b", a=2),
                        acc["q"][:])
                    nc.vector.tensor_copy(
                        kT[:, cs].rearrange("p (a b) -> p a b", a=2),
                        acc["k"][:])
                    acc["vtf"] = vtfp.tile([128, 2, CH], F32, tag="vtf",
                                           name=f"vtf{jp}")
                    nc.scalar.activation(acc["vtf"][:], acc["v"][:], COPYF)

                def evac_vt(half, j0=j0):
                    for tt in range(4):
                        tglob = 4 * (j0 + half) + tt
                        vt_ps = spsp.tile([128, 2, CH], F32, tag="sps",
                                          name=f"vt{tglob}")
                        nc.tensor.transpose(
                            vt_ps[:, 0, 0:128],
                            acc["vtf"][:, half, tt * 128:(tt + 1) * 128],
                            ident[:])
                        nc.scalar.activation(
                            vsb8[:, tglob // 2, :, tglob % 2, 0:DH],
                            vt_ps[:, 0, 0:128].rearrange(
                                "p (h c) -> p h c", c=DH),
                            COPYF)

                if jp == 0:
                    # d=0..3 only need the first halves of the weights, and
                    # the first matmul only wq; order so xt0 is 2nd in the
                    # (serial) DMA queue
                    nc.sync.dma_start(wq_sb[:, 0:CH], wq_d[:, 0:CH])
                dma_xt(0)
                if jp == 0:
                    nc.sync.dma_start(wk_sb[:, 0:CH], wk_d[:, 0:CH])
                    nc.sync.dma_start(wv_sb[:, 0:CH], wv_d[:, 0:CH])
                dma_xt(1)
                if jp == 0:
                    nc.sync.dma_start(wq_sb[:, CH:D], wq_d[:, CH:D])
                    nc.sync.dma_start(wk_sb[:, CH:D], wk_d[:, CH:D])
                    nc.sync.dma_start(wv_sb[:, CH:D], wv_d[:, CH:D])
                    nc.sync.dma_start(ones1[:], bc_d[:])
                for d in range(ND):
                    steps.append(lambda d=d: mm_step(d))
                steps.append(evac_qk)
                steps.append(lambda: evac_vt(0))
                steps.append(lambda: evac_vt(1))
                return steps

            for s in gen_proj(0):
                s()
            nc.sync.dma_start(wo_sb[:], wo_d[:])
            for mi in range(n_partial):
                nc.sync.dma_start(mask_sb[:, mi, :], mask_d[mi])

            for jp in range(NCH // 2):
                if jp + 1 < NCH // 2:
                    projq.extend(gen_proj(jp + 1))
                b = jp // 2
                for jj in (2 * (jp % 2), 2 * (jp % 2) + 1):
                    attention_chunk(b, jj)
                while projq:
                    projq.pop(0)()

            while norm_tail:
                norm_tail.pop(0)()
            while deferred:
                deferred.pop(0)()

    nc.compile()
    return nc


def _classify_mask(mask):
    """Classify (tq chunk, tk tile) blocks; build narrowed mask blocks."""
    maskT = mask.T  # [tk, tq]
    cats_key = []
    mask_tiles = []
    tile_index = {}
    for jj in range(NCH_B):
        first = True
        for i in range(NTK):
            blk = maskT[i * TK:(i + 1) * TK, jj * CH:(jj + 1) * CH]
            if blk.all():
                cats_key.append('f')
                first = False
                continue
            if not blk.any():
                cats_key.append('s')
                continue
            keepcol = blk.any(axis=0)  # col has any keep
            maskcol = (~blk).any(axis=0)  # col has any masked
            firstkeep = int(np.argmax(keepcol))
            c0 = 0 if first else min(256, 128 * (firstkeep // 128))
            first = False
            mcols = np.nonzero(maskcol)[0]
            mcols = mcols[mcols >= c0]
            if len(mcols) == 0:
                if c0 == 0:
                    cats_key.append('f')  # nothing masked at all
                else:
                    cats_key.append((-1, c0, 0, 0))  # narrowed, no add
                continue
            wa, wb = int(mcols[0]), int(mcols[-1]) + 1
            if wb - wa > MW:
                # fall back to a full-window add in 256 chunks; for causal
                # masks this never triggers
                wa, c0 = c0, c0
                wb = min(CH, wa + MW)
            key = blk[:, wa:wb].tobytes()
            if key not in tile_index:
                tile_index[key] = len(mask_tiles)
                m = np.where(blk[:, wa:wb], 0.0, NEG).astype(np.float32)
                mw = np.zeros((TK, MW), np.float32)
                mw[:, 0:wb - wa] = m
                mask_tiles.append(mw)
            cats_key.append((tile_index[key], c0, wa, wb))
    return tuple(cats_key), mask_tiles


def kernel(x, Wq, Wk, Wv, Wo, attn_mask):
    import concourse.bass_utils as _bu
    run_bass_kernel_spmd = _bu.run_bass_kernel_spmd

    x = np.asarray(x, dtype=np.float32)
    Wq = np.asarray(Wq, dtype=np.float32)
    Wk = np.asarray(Wk, dtype=np.float32)
    Wv = np.asarray(Wv, dtype=np.float32)
    Wo = np.asarray(Wo, dtype=np.float32)
    mask = np.asarray(attn_mask).astype(bool)

    xT = np.ascontiguousarray(x.reshape(BT, D).T)

    import ml_dtypes
    cats_key, mask_tiles = _classify_mask(mask)
    n_partial = len(mask_tiles)
    if n_partial:
        mask_arr = np.stack(mask_tiles)  # [n, 128, MW]
    else:
        mask_arr = np.zeros((1, TK, MW), np.float32)
    mask_arr = np.ascontiguousarray(mask_arr.astype(ml_dtypes.bfloat16))

    key = cats_key
    if key not in _cache:
        _cache[key] = _build(cats_key, n_partial)
    nc = _cache[key]

    in_maps = []
    for c in range(NCORES):
        rows = slice(c * DV, (c + 1) * DV)

        def wlayout(W, scale=1.0):
            Wc = W[rows, :]  # [128, D]
            return np.ascontiguousarray(
                (Wc.T.reshape(ND, 128, 128).transpose(1, 0, 2)
                 .reshape(128, D) * scale).astype(np.float32))

        wo_dev = np.ascontiguousarray(Wo[:, rows].T.astype(np.float32))
        bc_arr = np.ones((1, 128), np.float32)
        in_maps.append({
            "xt": xT,
            "wq": wlayout(Wq, 0.125),
            "wk": wlayout(Wk),
            "wv": wlayout(Wv),
            "wo": wo_dev,
            "mask": mask_arr,
            "bc": bc_arr,
        })

    res = run_bass_kernel_spmd(nc, in_maps, core_ids=list(range(NCORES)))
    out = np.zeros((BT, D), dtype=np.float32)
    for c in range(NCORES):
        out += np.asarray(res.results[c]["o"], dtype=np.float32)
    return out.reshape(B, T, D)


# revision 61
# speedup vs baseline: 1.0176x; 1.0176x over previous
"""Multi-head attention (B=2, T=2048, D=1024, 16 heads) on 8 TRN2 NeuronCores.

Sharding: tensor-parallel over heads (2 heads/core). Each core computes
Q/K/V projections for its 2 heads (full sequence), causal attention in
the S^T = K @ Q^T form (so attn @ V needs no transposes), and a partial
output projection o_c = attn_out_c @ Wo[:, cols_c].T in bf16. The host
sums the 8 partial [4096, 1024] outputs (the tensor-parallel all-reduce
done on host) and reshapes to [2, 2048, 1024].

Optimizations vs the original baseline (~347us -> ~210us):
- persistent PSUM pools: one 3-slot pool of [128,2,512] tiles shared by
  projection accumulators, score tiles, V-transposes, o-proj and the
  denominator broadcast, plus a 2-bank AV pool -- no per-phase pool
  scopes, so there are no scope-close barriers between phases.
- each chunk-pair's projection is emitted as small steps (1 d-step = 6
  matmuls + a prefetch DMA) pumped into the PREVIOUS pair's attention
  loop, so the PE stream never sees a phase boundary and xt DMAs are
  spread in time.
- softmax denominator: the ones-row of V lands the per-head denominators
  on PSUM partition 64; they are broadcast across partitions with two
  K=1 ones-matmuls and inverted with reciprocal_approx_fast on all 128
  lanes; no DRAM bounce (the old version did 5 DMA round-trips/chunk).
- causal-diagonal tiles narrow their score matmul / exp / AV windows to
  the live column range; the additive mask is applied by a cheap bf16
  PE matmul (identity^T @ maskblock accumulated into the score PSUM)
  instead of Vector adds.
- o partials written in bf16 (halves the 16MB output stream and the
  PSUM-evacuation copies); host sums partials in f32.
- normalization deferred into the next chunk and o-proj deferred into
  later chunks so PE work always exists between dependency chains.
- startup: weight DMAs split/ordered so the first matmul waits only on
  wq[:,0:512]+xt0; a few identity matmuls warm the PE p-state while the
  first DMAs are in flight.
"""

import sys

sys.path.insert(0, "/opt/trn_rl_repo")

import numpy as np

B, T, D = 2, 2048, 1024
NCORES = 8
DV = 128  # head dims per core (2 heads x 64)
DH = 64
BT = B * T
CH = 512  # tq chunk width
NCH = BT // CH  # 8 global chunks
NCH_B = T // CH  # 4 chunks per batch
TK = 128  # tk tile
NTK = T // TK  # 16 tiles per batch
ND = D // 128  # 8 contraction tiles
DVA = DH + 1  # V columns incl ones
NEG = -1.0e30
MW = 256  # max mask block width

_cache = {}


def _build(cats_key, n_partial):
    """Build + compile the SPMD Bass kernel for a given mask block structure.

    cats_key: tuple over (jj, i) of 'f' (full), 's' (skip), or a tuple
    (midx, c0, wa, wb): partial with mask block midx, score/exp window
    starting at column c0, mask add window [wa, wb).
    """
    import concourse.bacc as bacc
    import concourse.mybir as mybir
    import concourse.tile as tile
    from concourse.masks import make_identity

    F32 = mybir.dt.float32
    F32R = mybir.dt.float32r
    BF16 = mybir.dt.bfloat16
    FP8 = mybir.dt.float8e5  # unused
    EXP = mybir.ActivationFunctionType.Exp
    COPYF = mybir.ActivationFunctionType.Copy
    MULT = mybir.AluOpType.mult
    ADD = mybir.AluOpType.add
    DR = mybir.MatmulPerfMode.DoubleRow
    # exp() is biased by -ln(16) so p fits fp8e4m3 (max 448) for scores up
    # to ~8.9; the scale cancels between the AV numerator and the ones-row
    # denominator.
    EXPB = -2.772588722239781

    cats = {}
    idx = 0
    for jj in range(NCH_B):
        for i in range(NTK):
            cats[(jj, i)] = cats_key[idx]
            idx += 1

    nc = bacc.Bacc("TRN2", target_bir_lowering=False, debug=False, num_devices=NCORES)

    xt_d = nc.dram_tensor("xt", [D, BT], F32R, kind="ExternalInput").ap()
    wq_d = nc.dram_tensor("wq", [128, D], F32R, kind="ExternalInput").ap()
    wk_d = nc.dram_tensor("wk", [128, D], F32R, kind="ExternalInput").ap()
    wv_d = nc.dram_tensor("wv", [128, D], F32R, kind="ExternalInput").ap()
    wo_d = nc.dram_tensor("wo", [128, D], F32R, kind="ExternalInput").ap()
    nmask = max(n_partial, 1)
    mask_d = nc.dram_tensor("mask", [nmask, 128, MW], BF16, kind="ExternalInput").ap()
    bc_d = nc.dram_tensor("bc", [1, 128], F32R, kind="ExternalInput").ap()
    o_d = nc.dram_tensor("o", [BT, D], BF16, kind="ExternalOutput").ap()

    with tile.TileContext(nc) as tc:
        with tc.tile_pool(name="consts", bufs=1) as consts, \
             tc.tile_pool(name="perm", bufs=1) as perm, \
             tc.tile_pool(name="xt_pool", bufs=6) as xtp, \
             tc.tile_pool(name="vtf_pool", bufs=3) as vtfp, \
             tc.tile_pool(name="p_pool", bufs=4) as ppool, \
             tc.tile_pool(name="outT_pool", bufs=3) as outTp, \
             tc.tile_pool(name="rec_pool", bufs=3) as recp, \
             tc.tile_pool(name="osb_pool", bufs=3) as obp, \
             tc.tile_pool(name="sps_ps", bufs=3, space="PSUM") as spsp, \
             tc.tile_pool(name="av_ps", bufs=1, space="PSUM") as avp, \
             tc.tile_pool(name="dram_pool", bufs=2, space="DRAM") as drp:
            wq_sb = consts.tile([128, D], F32R, name="wq_sb")
            wk_sb = consts.tile([128, D], F32R, name="wk_sb")
            wv_sb = consts.tile([128, D], F32R, name="wv_sb")
            wo_sb = consts.tile([128, D], F32R, name="wo_sb")
            ident = consts.tile([128, 128], F32, name="ident")
            ones1 = consts.tile([1, 128], F32R, name="ones1")
            ones64 = consts.tile([128, 64, 1], F32, name="ones64")
            expb = consts.tile([128, 1], F32, name="expb")
            mask_sb = consts.tile([128, nmask, MW], BF16, name="mask_sb")
            identB = consts.tile([128, 128], BF16, name="identB")
            make_identity(nc, ident[:])
            nc.scalar.activation(identB[:], ident[:], COPYF)
            nc.gpsimd.memset(ones64[:], 1.0)
            nc.gpsimd.memset(expb[:], EXPB)

            qT = perm.tile([128, BT], F32R, name="qT")
            kT = perm.tile([128, BT], F32R, name="kT")
            # V blocks in fp8, DoubleRow pair-interleaved: per (tile-pair pi,
            # head h, tile-in-pair j) a [128(tk), 65] block (64 dims + ones
            # col). DR lhsT slice is vsb8[:, pi, h, :, :] = [128, 2, 65].
            NPAIR = B * NTK // 2
            vsb8 = perm.tile([128, NPAIR, 2, 2, DVA], F32R, name="vsb8")
            nc.scalar.activation(
                vsb8[:].rearrange("p a h j d -> p (a h j) d")[:, :, DH:DVA],
                ones64[:], COPYF)

            # warm the PE while the first DMAs are in flight: a few cheap
            # fp32 matmuls on the identity ramp the p-state before xt0 lands
            warm = spsp.tile([128, 2, CH], F32, tag="sps", name="warm")
            for _ in range(5):
                nc.tensor.matmul(warm[:, 0, 0:128], ident[:], ident[:],
                                 start=True, stop=True)

            deferred = []

            def emit_oproj(b, jj, outT):
                for tt in range(4):
                    def step(tt=tt, b=b, jj=jj, outT=outT):
                        op = spsp.tile([128, 2, CH], F32, tag="sps",
                                       name=f"op{b}_{jj}_{tt}")
                        ts = slice(tt * 128, (tt + 1) * 128)
                        nc.tensor.matmul(op[:, 0, :], outT[:, ts],
                                         wo_sb[:, 0:CH], start=True, stop=True)
                        nc.tensor.matmul(op[:, 1, :], outT[:, ts],
                                         wo_sb[:, CH:D], start=True, stop=True)
                        osb = obp.tile([128, D], BF16, tag="osb",
                                       name=f"osb{b}_{jj}_{tt}")
                        nc.vector.tensor_copy(
                            osb[:].rearrange("p (a b) -> p a b", a=2), op[:])
                        r0 = b * T + jj * CH + tt * 128
                        nc.sync.dma_start(o_d[r0:r0 + 128, :], osb[:])
                    deferred.append(step)

            norm_tail = []
            projq = []

            def pump(n):
                for _ in range(n):
                    if projq:
                        projq.pop(0)()

            def attention_chunk(b, jj):
                kept = [i for i in range(NTK) if cats[(jj, i)] != 's']
                if not kept:
                    return
                av = avp.tile([128, 2, CH], F32, tag="av", name=f"av_{b}_{jj}")
                tq0 = (b * NCH_B + jj) * CH
                pend = None

                def c0_of(i):
                    c = cats[(jj, i)]
                    if c == 'f' or i == kept[0]:
                        return 0
                    return c[1]

                def emit_av(unit):
                    kind, tiles, p, c0 = unit
                    st = kept[0] in tiles
                    sp = kept[-1] in tiles
                    if kind == 'dr':
                        pi = (b * NTK + tiles[0]) // 2
                        for h in (0, 1):
                            nc.tensor.matmul(
                                av[0:DVA, h, :], vsb8[:, pi, h, :, 0:DVA],
                                p[:, h, :, :], start=st, stop=sp,
                                perf_mode=DR)
                    else:
                        u = b * NTK + tiles[0]
                        for h in (0, 1):
                            nc.tensor.matmul(
                                av[0:DVA, h, c0:CH],
                                vsb8[:, u // 2, h, u % 2, 0:DVA],
                                p[:, h, c0:CH], start=st, stop=sp)

                pairs = [kept[x:x + 2] for x in range(0, len(kept), 2)]
                for pidx, pair in enumerate(pairs):
                    dr = False and (len(pair) == 2 and pair[1] == pair[0] + 1
                          and pair[0] % 2 == 0
                          and all(cats[(jj, i)] == 'f' for i in pair))
                    group = []
                    for i in pair:
                        c0 = c0_of(i)
                        ks = slice((b * NTK + i) * TK, (b * NTK + i + 1) * TK)
                        tqs = slice(tq0 + c0, tq0 + CH)
                        sps = spsp.tile([128, 2, CH], F32, tag="sps",
                                        name=f"sps{b}_{jj}_{i}")
                        c = cats[(jj, i)]
                        masked = c != 'f' and c[0] >= 0
                        for h in (0, 1):
                            nc.tensor.matmul(sps[:, h, c0:CH],
                                             kT[h * 64:(h + 1) * 64, ks],
                                             qT[h * 64:(h + 1) * 64, tqs],
                                             start=True, stop=not masked)
                        if masked:
                            # additive causal mask as a PE matmul:
                            # sps[:, h, wa:wb] += I^T @ maskblock (bf16)
                            midx, _, wa, wb = c
                            for h in (0, 1):
                                nc.tensor.matmul(
                                    sps[:, h, wa:wb], identB[:],
                                    mask_sb[:, midx, 0:wb - wa],
                                    start=False, stop=True)
                        group.append((i, sps, c0))
                    units = []
                    if dr:
                        p8 = ppool.tile([128, 2, 2, CH], F32R, tag="p8",
                                        name=f"p8_{b}_{jj}_{pair[0]}")
                        for (i, sps, c0), j in zip(group, (0, 1)):
                            nc.scalar.activation(p8[:, :, j, :], sps[:],
                                                 EXP)
                        units.append(('dr', pair, p8, 0))
                    else:
                        for i, sps, c0 in group:
                            p = ppool.tile([128, 2, CH], F32R, tag="p1",
                                           name=f"p{b}_{jj}_{i}")
                            nc.scalar.activation(p[:, :, c0:CH],
                                                 sps[:, :, c0:CH],
                                                 EXP)
                            units.append(('sg', (i,), p, c0))
                    if norm_tail and pidx == 0:
                        norm_tail.pop(0)()
                    if deferred and pidx >= 1:
                        deferred.pop(0)()
                        if len(deferred) > 4:
                            deferred.pop(0)()
                    if pend is not None:
                        for unit in pend:
                            emit_av(unit)
                    pend = units
                    pump(2)
                for unit in pend:
                    emit_av(unit)

                def norm_step(b=b, jj=jj, av=av):
                    # normalization: den rows live at partition 64 of both
                    # av banks; broadcast across partitions with two K=1
                    # ones-matmuls, then approx-reciprocal on all lanes
                    su = recp.tile([1, 2, CH], F32R, tag="su",
                                   name=f"su_{b}_{jj}")
                    nc.vector.tensor_copy(su[:], av[DH:DVA, :, :])
                    denb = spsp.tile([128, 2, CH], F32, tag="sps",
                                     name=f"denb_{b}_{jj}")
                    nc.tensor.matmul(denb[:, 0, :], ones1[:], su[:, 0, :],
                                     start=True, stop=True)
                    nc.tensor.matmul(denb[:, 1, :], ones1[:], su[:, 1, :],
                                     start=True, stop=True)
                    rdc = recp.tile([128, 2, CH], F32, tag="rdc",
                                    name=f"rdc{b}_{jj}")
                    nc.vector.reciprocal_approx_fast(out=rdc[:],
                                                     in_=denb[:])
                    outT = outTp.tile([128, CH], F32R, tag="outT",
                                      name=f"outT{b}_{jj}")
                    for h in (0, 1):
                        nc.vector.tensor_tensor(
                            out=outT[h * 64:(h + 1) * 64, :],
                            in0=av[0:DH, h, :],
                            in1=rdc[h * 64:(h + 1) * 64, h, :], op=MULT)
                    emit_oproj(b, jj, outT)
                norm_tail.append(norm_step)

            # ------- projection for chunk-pair jp, emitted as steps that
            # interleave into the previous pair's attention ----------------
            def gen_proj(jp):
                j0 = 2 * jp
                acc = {}
                xts = {}
                steps = []

                def dma_xt(d, jp=jp, j0=j0):
                    xt = xtp.tile([128, 2 * CH], F32R, tag="xt",
                                  name=f"xt{jp}_{d}")
                    nc.sync.dma_start(
                        xt[:], xt_d[d * 128:(d + 1) * 128,
                                    j0 * CH:(j0 + 2) * CH])
                    xts[d] = xt

                def mm_step(d, jp=jp):
                    if d + 2 < ND:
                        dma_xt(d + 2)
                    if d == 0:
                        for nm in ("q", "k", "v"):
                            acc[nm] = spsp.tile([128, 2, CH], F32, tag="sps",
                                                name=f"{nm}ps{jp}")
                    st, sp = d == 0, d == ND - 1
                    ws = slice(d * 128, (d + 1) * 128)
                    for nm, w_sb in (("q", wq_sb), ("k", wk_sb),
                                     ("v", wv_sb)):
                        for half in (0, 1):
                            nc.tensor.matmul(
                                acc[nm][:, half, :], w_sb[:, ws],
                                xts[d][:, half * CH:(half + 1) * CH],
                                start=st, stop=sp)
                    xts.pop(d)

                def evac_qk(jp=jp, j0=j0):
                    cs = slice(j0 * CH, (j0 + 2) * CH)
                    nc.vector.tensor_copy(
                        qT[:, cs].rearrange("p (a b) -> p a b", a=2),
                        acc["q"][:])
                    nc.vector.tensor_copy(
                        kT[:, cs].rearrange("p (a b) -> p a b", a=2),
                        acc["k"][:])
                    acc["vtf"] = vtfp.tile([128, 2, CH], F32, tag="vtf",
                                           name=f"vtf{jp}")
                    nc.scalar.activation(acc["vtf"][:], acc["v"][:], COPYF)

                def evac_vt(half, j0=j0):
                    for tt in range(4):
                        tglob = 4 * (j0 + half) + tt
                        vt_ps = spsp.tile([128, 2, CH], F32, tag="sps",
                                          name=f"vt{tglob}")
                        nc.tensor.transpose(
                            vt_ps[:, 0, 0:128],
                            acc["vtf"][:, half, tt * 128:(tt + 1) * 128],
                            ident[:])
                        nc.scalar.activation(
                            vsb8[:, tglob // 2, :, tglob % 2, 0:DH],
                            vt_ps[:, 0, 0:128].rearrange(
                                "p (h c) -> p h c", c=DH),
                            COPYF)

                if jp == 0:
                    # d=0..3 only need the first halves of the weights, and
                    # the first matmul only wq; order so xt0 is 2nd in the
                    # (serial) DMA queue
                    nc.sync.dma_start(wq_sb[:, 0:CH], wq_d[:, 0:CH])
                dma_xt(0)
                if jp == 0:
                    nc.sync.dma_start(wk_sb[:, 0:CH], wk_d[:, 0:CH])
                    nc.sync.dma_start(wv_sb[:, 0:CH], wv_d[:, 0:CH])
                dma_xt(1)
                if jp == 0:
                    nc.sync.dma_start(wq_sb[:, CH:D], wq_d[:, CH:D])
                    nc.sync.dma_start(wk_sb[:, CH:D], wk_d[:, CH:D])
                    nc.sync.dma_start(wv_sb[:, CH:D], wv_d[:, CH:D])
                    nc.sync.dma_start(ones1[:], bc_d[:])
                for d in range(ND):
                    steps.append(lambda d=d: mm_step(d))
                steps.append(evac_qk)
                steps.append(lambda: evac_vt(0))
                steps.append(lambda: evac_vt(1))
                return steps

            for s in gen_proj(0):
                s()
            nc.sync.dma_start(wo_sb[:], wo_d[:])
            for mi in range(n_partial):
                nc.sync.dma_start(mask_sb[:, mi, :], mask_d[mi])

            for jp in range(NCH // 2):
                if jp + 1 < NCH // 2:
                    projq.extend(gen_proj(jp + 1))
                b = jp // 2
                for jj in (2 * (jp % 2), 2 * (jp % 2) + 1):
                    attention_chunk(b, jj)
                while projq:
                    projq.pop(0)()

            while norm_tail:
                norm_tail.pop(0)()
            while deferred:
                deferred.pop(0)()

    nc.compile()
    return nc


def _classify_mask(mask):
    """Classify (tq chunk, tk tile) blocks; build narrowed mask blocks."""
    maskT = mask.T  # [tk, tq]
    cats_key = []
    mask_tiles = []
    tile_index = {}
    for jj in range(NCH_B):
        first = True
        for i in range(NTK):
            blk = maskT[i * TK:(i + 1) * TK, jj * CH:(jj + 1) * CH]
            if blk.all():
                cats_key.append('f')
                first = False
                continue
            if not blk.any():
                cats_key.append('s')
                continue
            keepcol = blk.any(axis=0)  # col has any keep
            maskcol = (~blk).any(axis=0)  # col has any masked
            firstkeep = int(np.argmax(keepcol))
            c0 = 0 if first else min(256, 128 * (firstkeep // 128))
            first = False
            mcols = np.nonzero(maskcol)[0]
            mcols = mcols[mcols >= c0]
            if len(mcols) == 0:
                if c0 == 0:
                    cats_key.append('f')  # nothing masked at all
                else:
                    cats_key.append((-1, c0, 0, 0))  # narrowed, no add
                continue
            wa, wb = int(mcols[0]), int(mcols[-1]) + 1
            if wb - wa > MW:
                # fall back to a full-window add in 256 chunks; for causal
                # masks this never triggers
                wa, c0 = c0, c0
                wb = min(CH, wa + MW)
            key = blk[:, wa:wb].tobytes()
            if key not in tile_index:
                tile_index[key] = len(mask_tiles)
                m = np.where(blk[:, wa:wb], 0.0, NEG).astype(np.float32)
                mw = np.zeros((TK, MW), np.float32)
                mw[:, 0:wb - wa] = m
                mask_tiles.append(mw)
            cats_key.append((tile_index[key], c0, wa, wb))
    return tuple(cats_key), mask_tiles


def kernel(x, Wq, Wk, Wv, Wo, attn_mask):
    import concourse.bass_utils as _bu
    run_bass_kernel_spmd = _bu.run_bass_kernel_spmd

    x = np.asarray(x, dtype=np.float32)
    Wq = np.asarray(Wq, dtype=np.float32)
    Wk = np.asarray(Wk, dtype=np.float32)
    Wv = np.asarray(Wv, dtype=np.float32)
    Wo = np.asarray(Wo, dtype=np.float32)
    mask = np.asarray(attn_mask).astype(bool)

    xT = np.ascontiguousarray(x.reshape(BT, D).T)

    import ml_dtypes
    cats_key, mask_tiles = _classify_mask(mask)
    n_partial = len(mask_tiles)
    if n_partial:
        mask_arr = np.stack(mask_tiles)  # [n, 128, MW]
    else:
        mask_arr = np.zeros((1, TK, MW), np.float32)
    mask_arr = np.ascontiguousarray(mask_arr.astype(ml_dtypes.bfloat16))

    key = cats_key
    if key not in _cache:
        _cache[key] = _build(cats_key, n_partial)
    nc = _cache[key]

    in_maps = []
    for c in range(NCORES):
        rows = slice(c * DV, (c + 1) * DV)

        def wlayout(W, scale=1.0):
            Wc = W[rows, :]  # [128, D]
            return np.ascontiguousarray(
                (Wc.T.reshape(ND, 128, 128).transpose(1, 0, 2)
                 .reshape(128, D) * scale).astype(np.float32))

        wo_dev = np.ascontiguousarray(Wo[:, rows].T.astype(np.float32))
        bc_arr = np.ones((1, 128), np.float32)
        in_maps.append({
            "xt": xT,
            "wq": wlayout(Wq, 0.125),
            "wk": wlayout(Wk),
            "wv": wlayout(Wv),
            "wo": wo_dev,
            "mask": mask_arr,
            "bc": bc_arr,
        })

    res = run_bass_kernel_spmd(nc, in_maps, core_ids=list(range(NCORES)))
    out = np.zeros((BT, D), dtype=np.float32)
    for c in range(NCORES):
        out += np.asarray(res.results[c]["o"], dtype=np.float32)
    return out.reshape(B, T, D)


# revision 62
# speedup vs baseline: 1.0793x; 1.0606x over previous
"""Multi-head attention (B=2, T=2048, D=1024, 16 heads) on 8 TRN2 NeuronCores.

Sharding: tensor-parallel over heads (2 heads/core). Each core computes
Q/K/V projections for its 2 heads (full sequence), causal attention in
the S^T = K @ Q^T form (so attn @ V needs no transposes), and a partial
output projection o_c = attn_out_c @ Wo[:, cols_c].T in bf16. The host
sums the 8 partial [4096, 1024] outputs (the tensor-parallel all-reduce
done on host) and reshapes to [2, 2048, 1024].

Optimizations vs the original baseline (~347us -> ~210us):
- persistent PSUM pools: one 3-slot pool of [128,2,512] tiles shared by
  projection accumulators, score tiles, V-transposes, o-proj and the
  denominator broadcast, plus a 2-bank AV pool -- no per-phase pool
  scopes, so there are no scope-close barriers between phases.
- each chunk-pair's projection is emitted as small steps (1 d-step = 6
  matmuls + a prefetch DMA) pumped into the PREVIOUS pair's attention
  loop, so the PE stream never sees a phase boundary and xt DMAs are
  spread in time.
- softmax denominator: the ones-row of V lands the per-head denominators
  on PSUM partition 64; they are broadcast across partitions with two
  K=1 ones-matmuls and inverted with reciprocal_approx_fast on all 128
  lanes; no DRAM bounce (the old version did 5 DMA round-trips/chunk).
- causal-diagonal tiles narrow their score matmul / exp / AV windows to
  the live column range; the additive mask is applied by a cheap bf16
  PE matmul (identity^T @ maskblock accumulated into the score PSUM)
  instead of Vector adds.
- o partials written in bf16 (halves the 16MB output stream and the
  PSUM-evacuation copies); host sums partials in f32.
- normalization deferred into the next chunk and o-proj deferred into
  later chunks so PE work always exists between dependency chains.
- startup: weight DMAs split/ordered so the first matmul waits only on
  wq[:,0:512]+xt0; a few identity matmuls warm the PE p-state while the
  first DMAs are in flight.
"""

import sys

sys.path.insert(0, "/opt/trn_rl_repo")

import numpy as np

B, T, D = 2, 2048, 1024
NCORES = 8
DV = 128  # head dims per core (2 heads x 64)
DH = 64
BT = B * T
CH = 512  # tq chunk width
NCH = BT // CH  # 8 global chunks
NCH_B = T // CH  # 4 chunks per batch
TK = 128  # tk tile
NTK = T // TK  # 16 tiles per batch
ND = D // 128  # 8 contraction tiles
DVA = DH + 1  # V columns incl ones
NEG = -1.0e30
MW = 256  # max mask block width

_cache = {}


def _build(cats_key, n_partial):
    """Build + compile the SPMD Bass kernel for a given mask block structure.

    cats_key: tuple over (jj, i) of 'f' (full), 's' (skip), or a tuple
    (midx, c0, wa, wb): partial with mask block midx, score/exp window
    starting at column c0, mask add window [wa, wb).
    """
    import concourse.bacc as bacc
    import concourse.mybir as mybir
    import concourse.tile as tile
    from concourse.masks import make_identity

    F32 = mybir.dt.float32
    F32R = mybir.dt.float32r
    BF16 = mybir.dt.bfloat16
    FP8 = mybir.dt.float8e5  # unused
    EXP = mybir.ActivationFunctionType.Exp
    COPYF = mybir.ActivationFunctionType.Copy
    MULT = mybir.AluOpType.mult
    ADD = mybir.AluOpType.add
    DR = mybir.MatmulPerfMode.DoubleRow
    # exp() is biased by -ln(16) so p fits fp8e4m3 (max 448) for scores up
    # to ~8.9; the scale cancels between the AV numerator and the ones-row
    # denominator.
    EXPB = -2.772588722239781

    cats = {}
    idx = 0
    for jj in range(NCH_B):
        for i in range(NTK):
            cats[(jj, i)] = cats_key[idx]
            idx += 1

    nc = bacc.Bacc("TRN2", target_bir_lowering=False, debug=False, num_devices=NCORES)

    xt_d = nc.dram_tensor("xt", [D, BT], F32R, kind="ExternalInput").ap()
    wq_d = nc.dram_tensor("wq", [128, D], F32R, kind="ExternalInput").ap()
    wk_d = nc.dram_tensor("wk", [128, D], F32R, kind="ExternalInput").ap()
    wv_d = nc.dram_tensor("wv", [128, D], F32R, kind="ExternalInput").ap()
    wo_d = nc.dram_tensor("wo", [128, D], F32R, kind="ExternalInput").ap()
    nmask = max(n_partial, 1)
    mask_d = nc.dram_tensor("mask", [nmask, 128, MW], BF16, kind="ExternalInput").ap()
    bc_d = nc.dram_tensor("bc", [1, 128], F32R, kind="ExternalInput").ap()
    o_d = nc.dram_tensor("o", [BT, D], BF16, kind="ExternalOutput").ap()

    with tile.TileContext(nc) as tc:
        with tc.tile_pool(name="consts", bufs=1) as consts, \
             tc.tile_pool(name="perm", bufs=1) as perm, \
             tc.tile_pool(name="xt_pool", bufs=6) as xtp, \
             tc.tile_pool(name="vtf_pool", bufs=3) as vtfp, \
             tc.tile_pool(name="p_pool", bufs=4) as ppool, \
             tc.tile_pool(name="outT_pool", bufs=3) as outTp, \
             tc.tile_pool(name="rec_pool", bufs=3) as recp, \
             tc.tile_pool(name="osb_pool", bufs=3) as obp, \
             tc.tile_pool(name="sps_ps", bufs=3, space="PSUM") as spsp, \
             tc.tile_pool(name="av_ps", bufs=1, space="PSUM") as avp, \
             tc.tile_pool(name="dram_pool", bufs=2, space="DRAM") as drp:
            wq_sb = consts.tile([128, D], F32R, name="wq_sb")
            wk_sb = consts.tile([128, D], F32R, name="wk_sb")
            wv_sb = consts.tile([128, D], F32R, name="wv_sb")
            wo_sb = consts.tile([128, D], F32R, name="wo_sb")
            ident = consts.tile([128, 128], F32, name="ident")
            ones1 = consts.tile([1, 128], F32R, name="ones1")
            ones64 = consts.tile([128, 64, 1], F32, name="ones64")
            expb = consts.tile([128, 1], F32, name="expb")
            mask_sb = consts.tile([128, nmask, MW], BF16, name="mask_sb")
            identB = consts.tile([128, 128], BF16, name="identB")
            make_identity(nc, ident[:])
            nc.scalar.activation(identB[:], ident[:], COPYF)
            nc.gpsimd.memset(ones64[:], 1.0)
            nc.gpsimd.memset(expb[:], EXPB)

            qT = perm.tile([128, BT], F32R, name="qT")
            # kT stored as two zero-padded planes so the score matmul's
            # stationary operand is a full 128x128 (enables FWL): plane h
            # holds head h's dims in their native partitions, zeros in the
            # other 64 partitions (those multiply don't-care rhs rows)
            kT2 = perm.tile([128, 2, BT], F32R, name="kT2")
            nc.gpsimd.memset(kT2[64:128, 0, :].bitcast(F32), 0.0)
            nc.gpsimd.memset(kT2[0:64, 1, :].bitcast(F32), 0.0)
            # V blocks in fp8, DoubleRow pair-interleaved: per (tile-pair pi,
            # head h, tile-in-pair j) a [128(tk), 65] block (64 dims + ones
            # col). DR lhsT slice is vsb8[:, pi, h, :, :] = [128, 2, 65].
            NPAIR = B * NTK // 2
            vsb8 = perm.tile([128, NPAIR, 2, 2, 128], F32R, name="vsb8")
            nc.scalar.activation(
                vsb8[:].rearrange("p a h j d -> p (a h j) d")[:, :, DH:DVA],
                ones64[:], COPYF)

            # warm the PE while the first DMAs are in flight: a few cheap
            # fp32 matmuls on the identity ramp the p-state before xt0 lands
            warm = spsp.tile([128, 2, CH], F32, tag="sps", name="warm")
            for _ in range(5):
                nc.tensor.matmul(warm[:, 0, 0:128], ident[:], ident[:],
                                 start=True, stop=True)

            deferred = []

            def emit_oproj(b, jj, outT):
                for tt in range(4):
                    def step(tt=tt, b=b, jj=jj, outT=outT):
                        op = spsp.tile([128, 2, CH], F32, tag="sps",
                                       name=f"op{b}_{jj}_{tt}")
                        ts = slice(tt * 128, (tt + 1) * 128)
                        nc.tensor.matmul(op[:, 0, :], outT[:, ts],
                                         wo_sb[:, 0:CH], start=True, stop=True)
                        nc.tensor.matmul(op[:, 1, :], outT[:, ts],
                                         wo_sb[:, CH:D], start=True, stop=True)
                        osb = obp.tile([128, D], BF16, tag="osb",
                                       name=f"osb{b}_{jj}_{tt}")
                        nc.vector.tensor_copy(
                            osb[:].rearrange("p (a b) -> p a b", a=2), op[:])
                        r0 = b * T + jj * CH + tt * 128
                        nc.sync.dma_start(o_d[r0:r0 + 128, :], osb[:])
                    deferred.append(step)

            norm_tail = []
            projq = []

            def pump(n):
                for _ in range(n):
                    if projq:
                        projq.pop(0)()

            def attention_chunk(b, jj):
                kept = [i for i in range(NTK) if cats[(jj, i)] != 's']
                if not kept:
                    return
                av = avp.tile([128, 2, CH], F32, tag="av", name=f"av_{b}_{jj}")
                tq0 = (b * NCH_B + jj) * CH
                pend = None

                def c0_of(i):
                    c = cats[(jj, i)]
                    if c == 'f' or i == kept[0]:
                        return 0
                    return c[1]

                def emit_av(unit):
                    kind, tiles, p, c0 = unit
                    st = kept[0] in tiles
                    sp = kept[-1] in tiles
                    if kind == 'dr':
                        pi = (b * NTK + tiles[0]) // 2
                        for h in (0, 1):
                            nc.tensor.matmul(
                                av[0:DVA, h, :], vsb8[:, pi, h, :, 0:DVA],
                                p[:, h, :, :], start=st, stop=sp,
                                perf_mode=DR)
                    else:
                        u = b * NTK + tiles[0]
                        for h in (0, 1):
                            nc.tensor.matmul(
                                av[:, h, c0:CH],
                                vsb8[:, u // 2, h, u % 2, :],
                                p[:, h, c0:CH], start=st, stop=sp)

                pairs = [kept[x:x + 2] for x in range(0, len(kept), 2)]
                for pidx, pair in enumerate(pairs):
                    dr = False and (len(pair) == 2 and pair[1] == pair[0] + 1
                          and pair[0] % 2 == 0
                          and all(cats[(jj, i)] == 'f' for i in pair))
                    group = []
                    for i in pair:
                        c0 = c0_of(i)
                        ks = slice((b * NTK + i) * TK, (b * NTK + i + 1) * TK)
                        tqs = slice(tq0 + c0, tq0 + CH)
                        sps = spsp.tile([128, 2, CH], F32, tag="sps",
                                        name=f"sps{b}_{jj}_{i}")
                        c = cats[(jj, i)]
                        masked = c != 'f' and c[0] >= 0
                        for h in (0, 1):
                            nc.tensor.matmul(sps[:, h, c0:CH],
                                             kT2[:, h, ks],
                                             qT[:, tqs],
                                             start=True, stop=not masked)
                        if masked:
                            # additive causal mask as a PE matmul:
                            # sps[:, h, wa:wb] += I^T @ maskblock (bf16)
                            midx, _, wa, wb = c
                            for h in (0, 1):
                                nc.tensor.matmul(
                                    sps[:, h, wa:wb], identB[:],
                                    mask_sb[:, midx, 0:wb - wa],
                                    start=False, stop=True)
                        group.append((i, sps, c0))
                    units = []
                    if dr:
                        p8 = ppool.tile([128, 2, 2, CH], F32R, tag="p8",
                                        name=f"p8_{b}_{jj}_{pair[0]}")
                        for (i, sps, c0), j in zip(group, (0, 1)):
                            nc.scalar.activation(p8[:, :, j, :], sps[:],
                                                 EXP)
                        units.append(('dr', pair, p8, 0))
                    else:
                        for i, sps, c0 in group:
                            p = ppool.tile([128, 2, CH], F32R, tag="p1",
                                           name=f"p{b}_{jj}_{i}")
                            nc.scalar.activation(p[:, :, c0:CH],
                                                 sps[:, :, c0:CH],
                                                 EXP)
                            units.append(('sg', (i,), p, c0))
                    if norm_tail and pidx == 0:
                        norm_tail.pop(0)()
                    if deferred and pidx >= 1:
                        deferred.pop(0)()
                        if len(deferred) > 4:
                            deferred.pop(0)()
                    if pend is not None:
                        for unit in pend:
                            emit_av(unit)
                    pend = units
                    pump(2)
                for unit in pend:
                    emit_av(unit)

                def norm_step(b=b, jj=jj, av=av):
                    # normalization: den rows live at partition 64 of both
                    # av banks; broadcast across partitions with two K=1
                    # ones-matmuls, then approx-reciprocal on all lanes
                    su = recp.tile([1, 2, CH], F32R, tag="su",
                                   name=f"su_{b}_{jj}")
                    nc.vector.tensor_copy(su[:], av[DH:DVA, :, :])
                    denb = spsp.tile([128, 2, CH], F32, tag="sps",
                                     name=f"denb_{b}_{jj}")
                    nc.tensor.matmul(denb[:, 0, :], ones1[:], su[:, 0, :],
                                     start=True, stop=True)
                    nc.tensor.matmul(denb[:, 1, :], ones1[:], su[:, 1, :],
                                     start=True, stop=True)
                    rdc = recp.tile([128, 2, CH], F32, tag="rdc",
                                    name=f"rdc{b}_{jj}")
                    nc.vector.reciprocal_approx_fast(out=rdc[:],
                                                     in_=denb[:])
                    outT = outTp.tile([128, CH], F32R, tag="outT",
                                      name=f"outT{b}_{jj}")
                    for h in (0, 1):
                        nc.vector.tensor_tensor(
                            out=outT[h * 64:(h + 1) * 64, :],
                            in0=av[0:DH, h, :],
                            in1=rdc[h * 64:(h + 1) * 64, h, :], op=MULT)
                    emit_oproj(b, jj, outT)
                norm_tail.append(norm_step)

            # ------- projection for chunk-pair jp, emitted as steps that
            # interleave into the previous pair's attention ----------------
            def gen_proj(jp):
                j0 = 2 * jp
                acc = {}
                xts = {}
                steps = []

                def dma_xt(d, jp=jp, j0=j0):
                    xt = xtp.tile([128, 2 * CH], F32R, tag="xt",
                                  name=f"xt{jp}_{d}")
                    nc.sync.dma_start(
                        xt[:], xt_d[d * 128:(d + 1) * 128,
                                    j0 * CH:(j0 + 2) * CH])
                    xts[d] = xt

                def mm_step(d, jp=jp):
                    if d + 2 < ND:
                        dma_xt(d + 2)
                    if d == 0:
                        for nm in ("q", "k", "v"):
                            acc[nm] = spsp.tile([128, 2, CH], F32, tag="sps",
                                                name=f"{nm}ps{jp}")
                    st, sp = d == 0, d == ND - 1
                    ws = slice(d * 128, (d + 1) * 128)
                    for nm, w_sb in (("q", wq_sb), ("k", wk_sb),
                                     ("v", wv_sb)):
                        for half in (0, 1):
                            nc.tensor.matmul(
                                acc[nm][:, half, :], w_sb[:, ws],
                                xts[d][:, half * CH:(half + 1) * CH],
                                start=st, stop=sp)
                    xts.pop(d)

                def evac_qk(jp=jp, j0=j0):
                    cs = slice(j0 * CH, (j0 + 2) * CH)
                    nc.vector.tensor_copy(
                        qT[:, cs].rearrange("p (a b) -> p a b", a=2),
                        acc["q"][:])
                    nc.vector.tensor_copy(
                        kT2[0:64, 0, cs].rearrange("p (a b) -> p a b", a=2),
                        acc["k"][0:64, :, :])
                    nc.vector.tensor_copy(
                        kT2[64:128, 1, cs].rearrange("p (a b) -> p a b", a=2),
                        acc["k"][64:128, :, :])
                    acc["vtf"] = vtfp.tile([128, 2, CH], F32, tag="vtf",
                                           name=f"vtf{jp}")
                    nc.scalar.activation(acc["vtf"][:], acc["v"][:], COPYF)

                def evac_vt(half, j0=j0):
                    for tt in range(4):
                        tglob = 4 * (j0 + half) + tt
                        vt_ps = spsp.tile([128, 2, CH], F32, tag="sps",
                                          name=f"vt{tglob}")
                        nc.tensor.transpose(
                            vt_ps[:, 0, 0:128],
                            acc["vtf"][:, half, tt * 128:(tt + 1) * 128],
                            ident[:])
                        nc.scalar.activation(
                            vsb8[:, tglob // 2, :, tglob % 2, 0:DH],
                            vt_ps[:, 0, 0:128].rearrange(
                                "p (h c) -> p h c", c=DH),
                            COPYF)

                if jp == 0:
                    # d=0..3 only need the first halves of the weights, and
                    # the first matmul only wq; order so xt0 is 2nd in the
                    # (serial) DMA queue
                    nc.sync.dma_start(wq_sb[:, 0:CH], wq_d[:, 0:CH])
                dma_xt(0)
                if jp == 0:
                    nc.sync.dma_start(wk_sb[:, 0:CH], wk_d[:, 0:CH])
                    nc.sync.dma_start(wv_sb[:, 0:CH], wv_d[:, 0:CH])
                dma_xt(1)
                if jp == 0:
                    nc.sync.dma_start(wq_sb[:, CH:D], wq_d[:, CH:D])
                    nc.sync.dma_start(wk_sb[:, CH:D], wk_d[:, CH:D])
                    nc.sync.dma_start(wv_sb[:, CH:D], wv_d[:, CH:D])
                    nc.sync.dma_start(ones1[:], bc_d[:])
                for d in range(ND):
                    steps.append(lambda d=d: mm_step(d))
                steps.append(evac_qk)
                steps.append(lambda: evac_vt(0))
                steps.append(lambda: evac_vt(1))
                return steps

            for s in gen_proj(0):
                s()
            nc.sync.dma_start(wo_sb[:], wo_d[:])
            for mi in range(n_partial):
                nc.sync.dma_start(mask_sb[:, mi, :], mask_d[mi])

            for jp in range(NCH // 2):
                if jp + 1 < NCH // 2:
                    projq.extend(gen_proj(jp + 1))
                b = jp // 2
                for jj in (2 * (jp % 2), 2 * (jp % 2) + 1):
                    attention_chunk(b, jj)
                while projq:
                    projq.pop(0)()

            while norm_tail:
                norm_tail.pop(0)()
            while deferred:
                deferred.pop(0)()

    nc.compile()
    return nc


def _classify_mask(mask):
    """Classify (tq chunk, tk tile) blocks; build narrowed mask blocks."""
    maskT = mask.T  # [tk, tq]
    cats_key = []
    mask_tiles = []
    tile_index = {}
    for jj in range(NCH_B):
        first = True
        for i in range(NTK):
            blk = maskT[i * TK:(i + 1) * TK, jj * CH:(jj + 1) * CH]
            if blk.all():
                cats_key.append('f')
                first = False
                continue
            if not blk.any():
                cats_key.append('s')
                continue
            keepcol = blk.any(axis=0)  # col has any keep
            maskcol = (~blk).any(axis=0)  # col has any masked
            firstkeep = int(np.argmax(keepcol))
            c0 = 0 if first else min(256, 128 * (firstkeep // 128))
            first = False
            mcols = np.nonzero(maskcol)[0]
            mcols = mcols[mcols >= c0]
            if len(mcols) == 0:
                if c0 == 0:
                    cats_key.append('f')  # nothing masked at all
                else:
                    cats_key.append((-1, c0, 0, 0))  # narrowed, no add
                continue
            wa, wb = int(mcols[0]), int(mcols[-1]) + 1
            if wb - wa > MW:
                # fall back to a full-window add in 256 chunks; for causal
                # masks this never triggers
                wa, c0 = c0, c0
                wb = min(CH, wa + MW)
            key = blk[:, wa:wb].tobytes()
            if key not in tile_index:
                tile_index[key] = len(mask_tiles)
                m = np.where(blk[:, wa:wb], 0.0, NEG).astype(np.float32)
                mw = np.zeros((TK, MW), np.float32)
                mw[:, 0:wb - wa] = m
                mask_tiles.append(mw)
            cats_key.append((tile_index[key], c0, wa, wb))
    return tuple(cats_key), mask_tiles


def kernel(x, Wq, Wk, Wv, Wo, attn_mask):
    import concourse.bass_utils as _bu
    run_bass_kernel_spmd = _bu.run_bass_kernel_spmd

    x = np.asarray(x, dtype=np.float32)
    Wq = np.asarray(Wq, dtype=np.float32)
    Wk = np.asarray(Wk, dtype=np.float32)
    Wv = np.asarray(Wv, dtype=np.float32)
    Wo = np.asarray(Wo, dtype=np.float32)
    mask = np.asarray(attn_mask).astype(bool)

    xT = np.ascontiguousarray(x.reshape(BT, D).T)

    import ml_dtypes
    cats_key, mask_tiles = _classify_mask(mask)
    n_partial = len(mask_tiles)
    if n_partial:
        mask_arr = np.stack(mask_tiles)  # [n, 128, MW]
    else:
        mask_arr = np.zeros((1, TK, MW), np.float32)
    mask_arr = np.ascontiguousarray(mask_arr.astype(ml_dtypes.bfloat16))

    key = cats_key
    if key not in _cache:
        _cache[key] = _build(cats_key, n_partial)
    nc = _cache[key]

    in_maps = []
    for c in range(NCORES):
        rows = slice(c * DV, (c + 1) * DV)

        def wlayout(W, scale=1.0):
            Wc = W[rows, :]  # [128, D]
            return np.ascontiguousarray(
                (Wc.T.reshape(ND, 128, 128).transpose(1, 0, 2)
                 .reshape(128, D) * scale).astype(np.float32))

        wo_dev = np.ascontiguousarray(Wo[:, rows].T.astype(np.float32))
        bc_arr = np.ones((1, 128), np.float32)
        in_maps.append({
            "xt": xT,
            "wq": wlayout(Wq, 0.125),
            "wk": wlayout(Wk),
            "wv": wlayout(Wv),
            "wo": wo_dev,
            "mask": mask_arr,
            "bc": bc_arr,
        })

    res = run_bass_kernel_spmd(nc, in_maps, core_ids=list(range(NCORES)))
    out = np.zeros((BT, D), dtype=np.float32)
    for c in range(NCORES):
        out += np.asarray(res.results[c]["o"], dtype=np.float32)
    return out.reshape(B, T, D)


# revision 63
# speedup vs baseline: 1.0859x; 1.0061x over previous
"""Multi-head attention (B=2, T=2048, D=1024, 16 heads) on 8 TRN2 NeuronCores.

Sharding: tensor-parallel over heads (2 heads/core). Each core computes
Q/K/V projections for its 2 heads (full sequence), causal attention in
the S^T = K @ Q^T form (so attn @ V needs no transposes), and a partial
output projection o_c = attn_out_c @ Wo[:, cols_c].T in bf16. The host
sums the 8 partial [4096, 1024] outputs (the tensor-parallel all-reduce
done on host) and reshapes to [2, 2048, 1024].

Optimizations vs the original baseline (~347us -> ~210us):
- persistent PSUM pools: one 3-slot pool of [128,2,512] tiles shared by
  projection accumulators, score tiles, V-transposes, o-proj and the
  denominator broadcast, plus a 2-bank AV pool -- no per-phase pool
  scopes, so there are no scope-close barriers between phases.
- each chunk-pair's projection is emitted as small steps (1 d-step = 6
  matmuls + a prefetch DMA) pumped into the PREVIOUS pair's attention
  loop, so the PE stream never sees a phase boundary and xt DMAs are
  spread in time.
- softmax denominator: the ones-row of V lands the per-head denominators
  on PSUM partition 64; they are broadcast across partitions with two
  K=1 ones-matmuls and inverted with reciprocal_approx_fast on all 128
  lanes; no DRAM bounce (the old version did 5 DMA round-trips/chunk).
- causal-diagonal tiles narrow their score matmul / exp / AV windows to
  the live column range; the additive mask is applied by a cheap bf16
  PE matmul (identity^T @ maskblock accumulated into the score PSUM)
  instead of Vector adds.
- o partials written in bf16 (halves the 16MB output stream and the
  PSUM-evacuation copies); host sums partials in f32.
- normalization deferred into the next chunk and o-proj deferred into
  later chunks so PE work always exists between dependency chains.
- startup: weight DMAs split/ordered so the first matmul waits only on
  wq[:,0:512]+xt0; a few identity matmuls warm the PE p-state while the
  first DMAs are in flight.
"""

import sys

sys.path.insert(0, "/opt/trn_rl_repo")

import numpy as np

B, T, D = 2, 2048, 1024
NCORES = 8
DV = 128  # head dims per core (2 heads x 64)
DH = 64
BT = B * T
CH = 512  # tq chunk width
NCH = BT // CH  # 8 global chunks
NCH_B = T // CH  # 4 chunks per batch
TK = 128  # tk tile
NTK = T // TK  # 16 tiles per batch
ND = D // 128  # 8 contraction tiles
DVA = DH + 1  # V columns incl ones
NEG = -1.0e30
MW = 256  # max mask block width

_cache = {}


def _build(cats_key, n_partial):
    """Build + compile the SPMD Bass kernel for a given mask block structure.

    cats_key: tuple over (jj, i) of 'f' (full), 's' (skip), or a tuple
    (midx, c0, wa, wb): partial with mask block midx, score/exp window
    starting at column c0, mask add window [wa, wb).
    """
    import concourse.bacc as bacc
    import concourse.mybir as mybir
    import concourse.tile as tile
    from concourse.masks import make_identity

    F32 = mybir.dt.float32
    F32R = mybir.dt.float32r
    BF16 = mybir.dt.bfloat16
    FP8 = mybir.dt.float8e5  # unused
    EXP = mybir.ActivationFunctionType.Exp
    COPYF = mybir.ActivationFunctionType.Copy
    MULT = mybir.AluOpType.mult
    ADD = mybir.AluOpType.add
    DR = mybir.MatmulPerfMode.DoubleRow
    # exp() is biased by -ln(16) so p fits fp8e4m3 (max 448) for scores up
    # to ~8.9; the scale cancels between the AV numerator and the ones-row
    # denominator.
    EXPB = -2.772588722239781

    cats = {}
    idx = 0
    for jj in range(NCH_B):
        for i in range(NTK):
            cats[(jj, i)] = cats_key[idx]
            idx += 1

    nc = bacc.Bacc("TRN2", target_bir_lowering=False, debug=False, num_devices=NCORES)

    xt_d = nc.dram_tensor("xt", [D, BT], F32R, kind="ExternalInput").ap()
    wq_d = nc.dram_tensor("wq", [128, D], F32R, kind="ExternalInput").ap()
    wk_d = nc.dram_tensor("wk", [128, D], F32R, kind="ExternalInput").ap()
    wv_d = nc.dram_tensor("wv", [128, D], F32R, kind="ExternalInput").ap()
    wo_d = nc.dram_tensor("wo", [128, D], F32R, kind="ExternalInput").ap()
    nmask = max(n_partial, 1)
    mask_d = nc.dram_tensor("mask", [nmask, 128, MW], BF16, kind="ExternalInput").ap()
    bc_d = nc.dram_tensor("bc", [1, 128], F32R, kind="ExternalInput").ap()
    o_d = nc.dram_tensor("o", [BT, D], BF16, kind="ExternalOutput").ap()

    with tile.TileContext(nc) as tc:
        with tc.tile_pool(name="consts", bufs=1) as consts, \
             tc.tile_pool(name="perm", bufs=1) as perm, \
             tc.tile_pool(name="xt_pool", bufs=6) as xtp, \
             tc.tile_pool(name="vtf_pool", bufs=3) as vtfp, \
             tc.tile_pool(name="p_pool", bufs=4) as ppool, \
             tc.tile_pool(name="outT_pool", bufs=3) as outTp, \
             tc.tile_pool(name="rec_pool", bufs=3) as recp, \
             tc.tile_pool(name="osb_pool", bufs=3) as obp, \
             tc.tile_pool(name="sps_ps", bufs=3, space="PSUM") as spsp, \
             tc.tile_pool(name="av_ps", bufs=1, space="PSUM") as avp, \
             tc.tile_pool(name="dram_pool", bufs=2, space="DRAM") as drp:
            wq_sb = consts.tile([128, D], F32R, name="wq_sb")
            wk_sb = consts.tile([128, D], F32R, name="wk_sb")
            wv_sb = consts.tile([128, D], F32R, name="wv_sb")
            wo_sb = consts.tile([128, D], F32R, name="wo_sb")
            ident = consts.tile([128, 128], F32, name="ident")
            ones1 = consts.tile([1, 128], F32R, name="ones1")
            ones64 = consts.tile([128, 64, 1], F32, name="ones64")
            expb = consts.tile([128, 1], F32, name="expb")
            mask_sb = consts.tile([128, nmask, MW], BF16, name="mask_sb")
            identB = consts.tile([128, 128], BF16, name="identB")
            make_identity(nc, ident[:])
            nc.scalar.activation(identB[:], ident[:], COPYF)
            nc.gpsimd.memset(ones64[:], 1.0)
            nc.gpsimd.memset(expb[:], EXPB)

            qT = perm.tile([128, BT], F32R, name="qT")
            # kT stored as two zero-padded planes so the score matmul's
            # stationary operand is a full 128x128 (enables FWL): plane h
            # holds head h's dims in their native partitions, zeros in the
            # other 64 partitions (those multiply don't-care rhs rows)
            kT2 = perm.tile([128, 2, BT], F32R, name="kT2")
            nc.gpsimd.memset(kT2[64:128, 0, :].bitcast(F32), 0.0)
            nc.gpsimd.memset(kT2[0:64, 1, :].bitcast(F32), 0.0)
            # V blocks in fp8, DoubleRow pair-interleaved: per (tile-pair pi,
            # head h, tile-in-pair j) a [128(tk), 65] block (64 dims + ones
            # col). DR lhsT slice is vsb8[:, pi, h, :, :] = [128, 2, 65].
            NPAIR = B * NTK // 2
            vsb8 = perm.tile([128, NPAIR, 2, 2, 128], F32R, name="vsb8")
            nc.scalar.activation(
                vsb8[:].rearrange("p a h j d -> p (a h j) d")[:, :, DH:DVA],
                ones64[:], COPYF)

            # warm the PE while the first DMAs are in flight: a few cheap
            # fp32 matmuls on the identity ramp the p-state before xt0 lands
            warm = spsp.tile([128, 2, CH], F32, tag="sps", name="warm")
            for _ in range(5):
                nc.tensor.matmul(warm[:, 0, 0:128], ident[:], ident[:],
                                 start=True, stop=True)

            deferred = []

            def emit_oproj(b, jj, outT):
                for tt in range(4):
                    def step(tt=tt, b=b, jj=jj, outT=outT):
                        op = spsp.tile([128, 2, CH], F32, tag="sps",
                                       name=f"op{b}_{jj}_{tt}")
                        ts = slice(tt * 128, (tt + 1) * 128)
                        nc.tensor.matmul(op[:, 0, :], outT[:, ts],
                                         wo_sb[:, 0:CH], start=True, stop=True)
                        nc.tensor.matmul(op[:, 1, :], outT[:, ts],
                                         wo_sb[:, CH:D], start=True, stop=True)
                        osb = obp.tile([128, D], BF16, tag="osb",
                                       name=f"osb{b}_{jj}_{tt}")
                        nc.vector.tensor_copy(
                            osb[:].rearrange("p (a b) -> p a b", a=2), op[:])
                        r0 = b * T + jj * CH + tt * 128
                        nc.sync.dma_start(o_d[r0:r0 + 128, :], osb[:])
                    deferred.append(step)

            norm_tail = []
            projq = []

            def pump(n):
                for _ in range(n):
                    if projq:
                        projq.pop(0)()

            def attention_chunk(b, jj):
                kept = [i for i in range(NTK) if cats[(jj, i)] != 's']
                if not kept:
                    return
                av = avp.tile([128, 2, CH], F32, tag="av", name=f"av_{b}_{jj}")
                tq0 = (b * NCH_B + jj) * CH
                pend = None

                def c0_of(i):
                    c = cats[(jj, i)]
                    if c == 'f' or i == kept[0]:
                        return 0
                    return c[1]

                def emit_av(unit):
                    kind, tiles, p, c0 = unit
                    st = kept[0] in tiles
                    sp = kept[-1] in tiles
                    if kind == 'dr':
                        pi = (b * NTK + tiles[0]) // 2
                        for h in (0, 1):
                            nc.tensor.matmul(
                                av[0:DVA, h, :], vsb8[:, pi, h, :, 0:DVA],
                                p[:, h, :, :], start=st, stop=sp,
                                perf_mode=DR)
                    else:
                        u = b * NTK + tiles[0]
                        for h in (0, 1):
                            nc.tensor.matmul(
                                av[:, h, c0:CH],
                                vsb8[:, u // 2, h, u % 2, :],
                                p[:, h, c0:CH], start=st, stop=sp)

                pairs = [kept[x:x + 2] for x in range(0, len(kept), 2)]
                for pidx, pair in enumerate(pairs):
                    dr = False and (len(pair) == 2 and pair[1] == pair[0] + 1
                          and pair[0] % 2 == 0
                          and all(cats[(jj, i)] == 'f' for i in pair))
                    group = []
                    for i in pair:
                        c0 = c0_of(i)
                        ks = slice((b * NTK + i) * TK, (b * NTK + i + 1) * TK)
                        tqs = slice(tq0 + c0, tq0 + CH)
                        sps = spsp.tile([128, 2, CH], F32, tag="sps",
                                        name=f"sps{b}_{jj}_{i}")
                        c = cats[(jj, i)]
                        masked = c != 'f' and c[0] >= 0
                        for h in (0, 1):
                            nc.tensor.matmul(sps[:, h, c0:CH],
                                             kT2[:, h, ks],
                                             qT[:, tqs],
                                             start=True, stop=not masked)
                        if masked:
                            # additive causal mask as a PE matmul:
                            # sps[:, h, wa:wb] += I^T @ maskblock (bf16)
                            midx, _, wa, wb = c
                            for h in (0, 1):
                                nc.tensor.matmul(
                                    sps[:, h, wa:wb], identB[:],
                                    mask_sb[:, midx, 0:wb - wa],
                                    start=False, stop=True)
                        group.append((i, sps, c0))
                    units = []
                    if dr:
                        p8 = ppool.tile([128, 2, 2, CH], F32R, tag="p8",
                                        name=f"p8_{b}_{jj}_{pair[0]}")
                        for (i, sps, c0), j in zip(group, (0, 1)):
                            nc.scalar.activation(p8[:, :, j, :], sps[:],
                                                 EXP)
                        units.append(('dr', pair, p8, 0))
                    else:
                        for i, sps, c0 in group:
                            p = ppool.tile([128, 2, CH], F32R, tag="p1",
                                           name=f"p{b}_{jj}_{i}")
                            nc.scalar.activation(p[:, :, c0:CH],
                                                 sps[:, :, c0:CH],
                                                 EXP)
                            units.append(('sg', (i,), p, c0))
                    if norm_tail and pidx == 0:
                        norm_tail.pop(0)()
                    if deferred and pidx >= 1:
                        deferred.pop(0)()
                        if len(deferred) > 6:
                            deferred.pop(0)()
                    if pend is not None:
                        for unit in pend:
                            emit_av(unit)
                    pend = units
                    pump(2)
                for unit in pend:
                    emit_av(unit)

                # den rows live at partition 64 of both av banks; extract
                # them immediately so the copy leads the DVE queue, while
                # the rest of the normalization is deferred a chunk
                su = recp.tile([1, 2, CH], F32R, tag="su",
                               name=f"su_{b}_{jj}")
                nc.vector.tensor_copy(su[:], av[DH:DVA, :, :])

                def norm_step(b=b, jj=jj, av=av, su=su):
                    # broadcast den across partitions with two K=1
                    # ones-matmuls, then approx-reciprocal on all lanes
                    denb = spsp.tile([128, 2, CH], F32, tag="sps",
                                     name=f"denb_{b}_{jj}")
                    nc.tensor.matmul(denb[:, 0, :], ones1[:], su[:, 0, :],
                                     start=True, stop=True)
                    nc.tensor.matmul(denb[:, 1, :], ones1[:], su[:, 1, :],
                                     start=True, stop=True)
                    rdc = recp.tile([128, 2, CH], F32, tag="rdc",
                                    name=f"rdc{b}_{jj}")
                    nc.vector.reciprocal_approx_fast(out=rdc[:],
                                                     in_=denb[:])
                    outT = outTp.tile([128, CH], F32R, tag="outT",
                                      name=f"outT{b}_{jj}")
                    for h in (0, 1):
                        nc.vector.tensor_tensor(
                            out=outT[h * 64:(h + 1) * 64, :],
                            in0=av[0:DH, h, :],
                            in1=rdc[h * 64:(h + 1) * 64, h, :], op=MULT)
                    emit_oproj(b, jj, outT)
                norm_tail.append(norm_step)

            # ------- projection for chunk-pair jp, emitted as steps that
            # interleave into the previous pair's attention ----------------
            def gen_proj(jp):
                j0 = 2 * jp
                acc = {}
                xts = {}
                steps = []

                def dma_xt(d, jp=jp, j0=j0):
                    xt = xtp.tile([128, 2 * CH], F32R, tag="xt",
                                  name=f"xt{jp}_{d}")
                    nc.sync.dma_start(
                        xt[:], xt_d[d * 128:(d + 1) * 128,
                                    j0 * CH:(j0 + 2) * CH])
                    xts[d] = xt

                def mm_step(d, jp=jp):
                    if d + 2 < ND:
                        dma_xt(d + 2)
                    if d == 0:
                        for nm in ("q", "k", "v"):
                            acc[nm] = spsp.tile([128, 2, CH], F32, tag="sps",
                                                name=f"{nm}ps{jp}")
                    st, sp = d == 0, d == ND - 1
                    ws = slice(d * 128, (d + 1) * 128)
                    for nm, w_sb in (("q", wq_sb), ("k", wk_sb),
                                     ("v", wv_sb)):
                        for half in (0, 1):
                            nc.tensor.matmul(
                                acc[nm][:, half, :], w_sb[:, ws],
                                xts[d][:, half * CH:(half + 1) * CH],
                                start=st, stop=sp)
                    xts.pop(d)

                def evac_qk(jp=jp, j0=j0):
                    cs = slice(j0 * CH, (j0 + 2) * CH)
                    nc.vector.tensor_copy(
                        qT[:, cs].rearrange("p (a b) -> p a b", a=2),
                        acc["q"][:])
                    nc.vector.tensor_copy(
                        kT2[0:64, 0, cs].rearrange("p (a b) -> p a b", a=2),
                        acc["k"][0:64, :, :])
                    nc.vector.tensor_copy(
                        kT2[64:128, 1, cs].rearrange("p (a b) -> p a b", a=2),
                        acc["k"][64:128, :, :])
                    acc["vtf"] = vtfp.tile([128, 2, CH], F32, tag="vtf",
                                           name=f"vtf{jp}")
                    nc.scalar.activation(acc["vtf"][:], acc["v"][:], COPYF)

                def evac_vt(half, j0=j0):
                    for tt in range(4):
                        tglob = 4 * (j0 + half) + tt
                        vt_ps = spsp.tile([128, 2, CH], F32, tag="sps",
                                          name=f"vt{tglob}")
                        nc.tensor.transpose(
                            vt_ps[:, 0, 0:128],
                            acc["vtf"][:, half, tt * 128:(tt + 1) * 128],
                            ident[:])
                        nc.scalar.activation(
                            vsb8[:, tglob // 2, :, tglob % 2, 0:DH],
                            vt_ps[:, 0, 0:128].rearrange(
                                "p (h c) -> p h c", c=DH),
                            COPYF)

                if jp == 0:
                    # d=0..3 only need the first halves of the weights, and
                    # the first matmul only wq; order so xt0 is 2nd in the
                    # (serial) DMA queue
                    nc.sync.dma_start(wq_sb[:, 0:CH], wq_d[:, 0:CH])
                dma_xt(0)
                if jp == 0:
                    nc.sync.dma_start(wk_sb[:, 0:CH], wk_d[:, 0:CH])
                    nc.sync.dma_start(wv_sb[:, 0:CH], wv_d[:, 0:CH])
                dma_xt(1)
                if jp == 0:
                    nc.sync.dma_start(wq_sb[:, CH:D], wq_d[:, CH:D])
                    nc.sync.dma_start(wk_sb[:, CH:D], wk_d[:, CH:D])
                    nc.sync.dma_start(wv_sb[:, CH:D], wv_d[:, CH:D])
                    nc.sync.dma_start(ones1[:], bc_d[:])
                for d in range(ND):
                    steps.append(lambda d=d: mm_step(d))
                steps.append(evac_qk)
                steps.append(lambda: evac_vt(0))
                steps.append(lambda: evac_vt(1))
                return steps

            for s in gen_proj(0):
                s()
            nc.sync.dma_start(wo_sb[:], wo_d[:])
            for mi in range(n_partial):
                nc.sync.dma_start(mask_sb[:, mi, :], mask_d[mi])

            for jp in range(NCH // 2):
                if jp + 1 < NCH // 2:
                    projq.extend(gen_proj(jp + 1))
                b = jp // 2
                for jj in (2 * (jp % 2), 2 * (jp % 2) + 1):
                    attention_chunk(b, jj)
                while projq:
                    projq.pop(0)()

            while norm_tail:
                norm_tail.pop(0)()
            while deferred:
                deferred.pop(0)()

    nc.compile()
    return nc


def _classify_mask(mask):
    """Classify (tq chunk, tk tile) blocks; build narrowed mask blocks."""
    maskT = mask.T  # [tk, tq]
    cats_key = []
    mask_tiles = []
    tile_index = {}
    for jj in range(NCH_B):
        first = True
        for i in range(NTK):
            blk = maskT[i * TK:(i + 1) * TK, jj * CH:(jj + 1) * CH]
            if blk.all():
                cats_key.append('f')
                first = False
                continue
            if not blk.any():
                cats_key.append('s')
                continue
            keepcol = blk.any(axis=0)  # col has any keep
            maskcol = (~blk).any(axis=0)  # col has any masked
            firstkeep = int(np.argmax(keepcol))
            c0 = 0 if first else min(256, 128 * (firstkeep // 128))
            first = False
            mcols = np.nonzero(maskcol)[0]
            mcols = mcols[mcols >= c0]
            if len(mcols) == 0:
                if c0 == 0:
                    cats_key.append('f')  # nothing masked at all
                else:
                    cats_key.append((-1, c0, 0, 0))  # narrowed, no add
                continue
            wa, wb = int(mcols[0]), int(mcols[-1]) + 1
            if wb - wa > MW:
                # fall back to a full-window add in 256 chunks; for causal
                # masks this never triggers
                wa, c0 = c0, c0
                wb = min(CH, wa + MW)
            key = blk[:, wa:wb].tobytes()
            if key not in tile_index:
                tile_index[key] = len(mask_tiles)
                m = np.where(blk[:, wa:wb], 0.0, NEG).astype(np.float32)
                mw = np.zeros((TK, MW), np.float32)
                mw[:, 0:wb - wa] = m
                mask_tiles.append(mw)
            cats_key.append((tile_index[key], c0, wa, wb))
    return tuple(cats_key), mask_tiles


def kernel(x, Wq, Wk, Wv, Wo, attn_mask):
    import concourse.bass_utils as _bu
    run_bass_kernel_spmd = _bu.run_bass_kernel_spmd

    x = np.asarray(x, dtype=np.float32)
    Wq = np.asarray(Wq, dtype=np.float32)
    Wk = np.asarray(Wk, dtype=np.float32)
    Wv = np.asarray(Wv, dtype=np.float32)
    Wo = np.asarray(Wo, dtype=np.float32)
    mask = np.asarray(attn_mask).astype(bool)

    xT = np.ascontiguousarray(x.reshape(BT, D).T)

    import ml_dtypes
    cats_key, mask_tiles = _classify_mask(mask)
    n_partial = len(mask_tiles)
    if n_partial:
        mask_arr = np.stack(mask_tiles)  # [n, 128, MW]
    else:
        mask_arr = np.zeros((1, TK, MW), np.float32)
    mask_arr = np.ascontiguousarray(mask_arr.astype(ml_dtypes.bfloat16))

    key = cats_key
    if key not in _cache:
        _cache[key] = _build(cats_key, n_partial)
    nc = _cache[key]

    in_maps = []
    for c in range(NCORES):
        rows = slice(c * DV, (c + 1) * DV)

        def wlayout(W, scale=1.0):
            Wc = W[rows, :]  # [128, D]
            return np.ascontiguousarray(
                (Wc.T.reshape(ND, 128, 128).transpose(1, 0, 2)
                 .reshape(128, D) * scale).astype(np.float32))

        wo_dev = np.ascontiguousarray(Wo[:, rows].T.astype(np.float32))
        bc_arr = np.ones((1, 128), np.float32)
        in_maps.append({
            "xt": xT,
            "wq": wlayout(Wq, 0.125),
            "wk": wlayout(Wk),
            "wv": wlayout(Wv),
            "wo": wo_dev,
            "mask": mask_arr,
            "bc": bc_arr,
        })

    res = run_bass_kernel_spmd(nc, in_maps, core_ids=list(range(NCORES)))
    out = np.zeros((BT, D), dtype=np.float32)
    for c in range(NCORES):
        out += np.asarray(res.results[c]["o"], dtype=np.float32)
    return out.reshape(B, T, D)
